# revision 1
# baseline (speedup 1.0000x reference)
"""Trainium2 Bass kernel for nn_LstmEncoder: two 5-layer LSTM stacks.

Architecture (hardcoded from the problem spec):
  x [256, 256, 8] -> stack1 (F=8 -> H=256, 5 layers) -> stack2 (H=256 -> E=128,
  5 layers) -> returns final hidden of last layer, [256, 128].

Sharding: data-parallel over batch, 32 rows per core on 8 cores; weights
replicated.  Per core, the 10 LSTM layers run as a diagonal wavefront
(layer q processes timestep t = s - q at wavefront step s); 4 layers' 32-row
matmuls pack into the 4 column groups of the PE array (tile_position, batch
on psum partitions 32j).

Key optimizations over the fp32 baseline (9.5 ms -> ~2.7 ms):
  - All gate matmuls in bf16 (PSUM accumulates fp32): the PE streams 1
    output row/cycle instead of 4 for fp32.
  - Pass-A biases move off the PE entirely: a full [128, 1024] per-row
    bias table is added to the psum gates with one DVE tensor_add per
    pass; each pass-A layer's first matmul per region carries start=True
    to reset its psum rows.  Pass B keeps one indicator matmul per region
    (lhsT [4, 128] one-hot rows x stacked biases, start=True) because it
    doubles as the zero-init of the pair-slot columns no layer matmul
    covers.
  - Software-pipelined emission: within step s the PE runs
    [A-class matmuls (consume step s-1 A-transposes) | B-indicator,
     q4-in | B-transposes of s-1 | B-class matmuls], so the pass-B
    elementwise latency chain of s-1 hides under pass-A streaming and
    the PE has ~0% idle in steady state.
  - h is bf16 (transposes cost 1 row-cycle); all hT psum->sbuf copies on
    the DVE.  The B copies were originally on the scalar engine, but the
    2KB-granular psum zero-region makes transpose ki+1 wait for ki's copy,
    and Act is busy with pass-A activations right then while the DVE is
    free — moving them measured 2.58 ms vs 2.85 ms head-to-head.

Rejected experiments (measured slower or infeasible):
  - fp8 DoubleRow matmuls: ISA requires psum partition base 0 for the
    output, so only rows-0:32 layers qualify; even restricted to those, HW
    measured 3.96 ms vs 2.76 ms (the cost model's 0.5 cycles/row for
    DoubleRow does not hold on silicon here).
  - Pass-B bias via DVE add as well (removing the B indicator): needs
    per-chunk start=True flags plus gap-column memsets, and measured
    4.58 ms vs 2.97 ms head-to-head — the added DVE work sits on the
    B-side latency chain that feeds the next step's B matmuls.
  - DMA of host-precomputed layer-0 input gates into PSUM: bass forbids
    PSUM as a DMA destination.
  - Pool-engine psum->sbuf copies: GPSIMD cannot access PSUM (walrus
    birverifier).
  - Spreading the 4 h-transposes across two psum banks in one bufs=1
    [128, 2048] tile (to dodge the 2KB-granular zero-region serialization
    the cost model shows): measured 5.09 ms vs 3.43 ms head-to-head —
    single-buffered cross-iteration tp writes hit the tile framework's
    conservative min-join sync.  Keep tppool bufs=2 [128, 512].
  - Dropping q0's K=9 input matmuls (-1024 PE rows/step) by staging
    host-precomputed xg0(t)+bias per step into SBUF for the pass-A DVE
    add: measured 3.06 ms vs 2.98 ms head-to-head — the two per-step
    staging DMAs and the add's new dependency cost more than the saved
    PE rows.
"""

import numpy as np
import ml_dtypes

BF16 = ml_dtypes.bfloat16

B, T_FULL, F, E = 256, 256, 8, 128
H = 2 * E          # 256
NL = 5
NCORES = 8
BSH = B // NCORES  # 32


def _gate_perm(Hd):
    """PyTorch gate order [i f g o] -> our column order [i f o g]."""
    return np.concatenate([
        np.arange(0, Hd),
        np.arange(Hd, 2 * Hd),
        np.arange(3 * Hd, 4 * Hd),
        np.arange(2 * Hd, 3 * Hd),
    ])


# ---------------- layer table helpers (chain index q = 0..9) ----------------
# q 0..4: stack1 (Hd=256, gates 1024); q 5..9: stack2 (Hd=128, gates 512).

def _is_s1(q):
    return q < 5


def _pass_of(q):
    return "A" if q < 4 else "B"


def _rows(q):
    """Partition row range of layer q within its pass tile."""
    if q < 4:
        return 32 * q, 32 * q + 32
    if q == 4:
        return 0, 32
    p, _slot = divmod(q - 5, 2)
    return 32 * (p + 1), 32 * (p + 2)


def _slot(q):
    return (q - 5) % 2 if q >= 5 else 0


def _ch_cols(q):
    """(c-block col range, h-block col range) within c_all/h_all [128, 512]."""
    if q < 4:
        return (0, 256)
    if q == 4:
        return (256, 512)
    return (256, 384) if _slot(q) == 0 else (384, 512)


def _own_hT(q):
    """List of (buf_key, col) giving this layer's hT stationaries (K-tiles)."""
    if q < 4:
        return [(("A", 0), 32 * q), (("A", 1), 32 * q)]
    if q == 4:
        return [(("B", 0), 0), (("B", 1), 0)]
    p, slot = divmod(q - 5, 2)
    return [(("B", slot), 32 * (p + 1))]


def _in_hT(q):
    if q == 0:
        return None  # x input (xT)
    if 1 <= q <= 4:
        return [(("A", 0), 32 * (q - 1)), (("A", 1), 32 * (q - 1))]
    if q == 5:
        return [(("B", 0), 0), (("B", 1), 0)]
    return _own_hT(q - 1)


def build_nc(T, smax=None):
    import concourse.bass as bass
    import concourse.mybir as mybir
    import concourse.tile as tile
    from concourse import bacc
    from contextlib import ExitStack

    fp = mybir.dt.float32
    bf = mybir.dt.bfloat16
    AF = mybir.ActivationFunctionType
    nc = bacc.Bacc("TRN2", target_bir_lowering=False)

    # ---------------- DRAM declarations ----------------
    xT_d = nc.dram_tensor("xT", [9, T * BSH], bf, kind="ExternalInput")
    id_d = nc.dram_tensor("ident", [128, 128], bf, kind="ExternalInput")
    win_d, whh_d = {}, {}
    for q in range(10):
        G = 1024 if _is_s1(q) else 512
        kt_in = 1 if q == 0 else (2 if (_is_s1(q) or q == 5) else 1)
        kt_hh = 2 if _is_s1(q) else 1
        for k in range(kt_in):
            kp = 9 if q == 0 else 128
            win_d[q, k] = nc.dram_tensor(f"win{q}_{k}", [kp, G], bf,
                                         kind="ExternalInput")
        for k in range(kt_hh):
            whh_d[q, k] = nc.dram_tensor(f"whh{q}_{k}", [128, G], bf,
                                         kind="ExternalInput")
    biasA_d = nc.dram_tensor("biasA", [128, 1024], bf, kind="ExternalInput")
    indB_d = nc.dram_tensor("indB", [4, 128], bf, kind="ExternalInput")
    bcatB_d = nc.dram_tensor("bcatB", [4, 1024], bf, kind="ExternalInput")
    out_d = nc.dram_tensor("out", [BSH, E], bf, kind="ExternalOutput")

    S = T + 9 if smax is None else smax + 1

    with tile.TileContext(nc) as tc, ExitStack() as ctx:
        wpool = ctx.enter_context(tc.tile_pool(name="weights", bufs=1))
        state = ctx.enter_context(tc.tile_pool(name="state", bufs=1))
        hpool = ctx.enter_context(tc.tile_pool(name="hpool", bufs=2))
        hTpool = ctx.enter_context(tc.tile_pool(name="hTpool", bufs=2))
        apool = ctx.enter_context(tc.tile_pool(name="apool", bufs=2))
        gApool = ctx.enter_context(tc.tile_pool(name="gApool", bufs=2,
                                                space="PSUM"))
        gBpool = ctx.enter_context(tc.tile_pool(name="gBpool", bufs=1,
                                                space="PSUM"))
        tppool = ctx.enter_context(tc.tile_pool(name="tppool", bufs=2,
                                                space="PSUM"))

        def load(dram, dtype):
            t = wpool.tile(list(dram.shape), dtype, name=f"sb_{dram.name}")
            nc.sync.dma_start(t[:], dram[:])
            return t

        xT_sb = load(xT_d, bf)
        id_sb = load(id_d, bf)
        win_sb = {k: load(v, bf) for k, v in win_d.items()}
        whh_sb = {k: load(v, bf) for k, v in whh_d.items()}
        biasA_sb = load(biasA_d, bf)
        indB_sb = load(indB_d, bf)
        bcatB_sb = load(bcatB_d, bf)

        c_all = state.tile([128, 512], fp, name="c_all")
        nc.gpsimd.memset(c_all[:], 0.0)

        hT = {}              # latest hT tiles by key ("A"/"B", k)
        h_prev = None        # h_tile of s-1 (bf16)
        tp_prev = None       # transpose psum tile of s-1
        prev_actB = []

        def emit_mms(seq):
            for o, lh, rh, st, sp, j in seq:
                nc.tensor.matmul(o, lh, rh, start=st, stop=sp,
                                 skip_group_check=True,
                                 tile_position=(0, 32 * j))

        def colgroup(q):
            if q < 4:
                return q
            if q == 4:
                return 0
            return (q - 5) // 2 + 1

        def layer_seq(q, s, g, in_part=True, rec_part=True):
            """Region-interleaved matmul list for layer q at wstep s."""
            t = s - q
            r0, r1 = _rows(q)
            j = colgroup(q)
            mms_r = {0: [], 1: []}
            if _is_s1(q):
                if in_part:
                    kt_in = 1 if q == 0 else 2
                    for k in range(kt_in):
                        if q == 0:
                            lh = xT_sb[:, 32 * t:32 * t + 32]
                        else:
                            (bk, col) = _in_hT(q)[k]
                            lh = hT[bk][:, col:col + 32]
                        for r, (n0, n1) in enumerate(((0, 512), (512, 1024))):
                            # pass-A layers self-initialize their psum rows;
                            # q4 (pass B) accumulates onto the B indicator
                            mms_r[r].append((g[r0:r1, n0:n1], lh,
                                             win_sb[q, k][:, n0:n1],
                                             k == 0 and q < 4))
                if rec_part and t > 0:
                    for k in range(2):
                        (bk, col) = _own_hT(q)[k]
                        lh = hT[bk][:, col:col + 32]
                        for r, (n0, n1) in enumerate(((0, 512), (512, 1024))):
                            mms_r[r].append((g[r0:r1, n0:n1], lh,
                                             whh_sb[q, k][:, n0:n1], False))
            else:
                p, slot = divmod(q - 5, 2)
                if in_part:
                    kt_in = 2 if q == 5 else 1
                    for k in range(kt_in):
                        (bk, col) = _in_hT(q)[k]
                        lh = hT[bk][:, col:col + 32]
                        for c in range(4):
                            o0 = 256 * c + 128 * slot
                            mms_r[c // 2].append((g[r0:r1, o0:o0 + 128], lh,
                                                  win_sb[q, k][:, 128 * c:128 * c + 128],
                                                  False))
                if rec_part and t > 0:
                    (bk, col) = _own_hT(q)[0]
                    lh = hT[bk][:, col:col + 32]
                    for c in range(4):
                        o0 = 256 * c + 128 * slot
                        mms_r[c // 2].append((g[r0:r1, o0:o0 + 128], lh,
                                              whh_sb[q, 0][:, 128 * c:128 * c + 128],
                                              False))
            seq = []
            for i in range(max(len(mms_r[0]), len(mms_r[1]))):
                for r in range(2):
                    if i < len(mms_r[r]):
                        o, lh, rh, st = mms_r[r][i]
                        last = i == len(mms_r[r]) - 1
                        seq.append((o, lh, rh, st, last, j))
            return seq

        for s in range(S):
            act = [q for q in range(10) if 0 <= s - q <= T - 1]
            actA = [q for q in act if _pass_of(q) == "A"]
            actB = [q for q in act if _pass_of(q) == "B"]

            for q in act:
                if s - q == 0:
                    r0, r1 = _rows(q)
                    cc0, cc1 = _ch_cols(q)
                    nc.gpsimd.memset(c_all[r0:r1, cc0:cc1], 0.0)

            gA = gApool.tile([128, 1024], fp, name="gA", tag="gA") if actA \
                else None
            gB = gBpool.tile([128, 1024], fp, name="gB", tag="gB") if actB \
                else None
            h_tile = hpool.tile([128, 512], bf, name="h", tag="h")
            tp_cur = tppool.tile([128, 512], bf, name="tp", tag="tp")

            # ---- PE: A-class matmuls (consume A hT of s-1; layer's first
            # matmul per region carries start=True to reset its psum rows)
            aseq = []
            for q in (0, 1, 2, 3):
                if q in act:
                    aseq.extend(layer_seq(q, s, gA))
            emit_mms(aseq)

            # ---- PE: B-indicator (bias + zero-init), q4-in (A hT of s-1)
            if actB:
                for n0, n1 in ((0, 512), (512, 1024)):
                    nc.tensor.matmul(gB[0:128, n0:n1], indB_sb[:, 0:128],
                                     bcatB_sb[:, n0:n1], start=True,
                                     stop=False, skip_group_check=True,
                                     tile_position=(0, 0))
            if 4 in act:
                emit_mms(layer_seq(4, s, gB, rec_part=False))

            # ---- PE: B-transposes of s-1 (+ DVE copies) -> new B hT tiles.
            # Copies on DVE, not Act: the 2KB-granular psum zero-region makes
            # transpose ki+1 wait for ki's copy, and at this point Act is
            # busy with pass-A activations while DVE has just the bias add.
            if prev_actB:
                for ki, k in ((2, 0), (3, 1)):
                    src = h_prev[:, 128 * ki:128 * ki + 128]
                    nc.tensor.transpose(tp_prev[:, 128 * ki:128 * ki + 128],
                                        src, id_sb[:])
                    dst = hTpool.tile([128, 128], bf, name=f"hTB{k}",
                                      tag=f"hTB{k}")
                    nc.vector.tensor_copy(dst[:],
                                          tp_prev[:, 128 * ki:128 * ki + 128])
                    hT[("B", k)] = dst

            # ---- PE: B-class matmuls
            bseq = []
            if 4 in act and s - 4 > 0:
                bseq.extend(layer_seq(4, s, gB, in_part=False))
            for q in range(5, 10):
                if q in act:
                    bseq.extend(layer_seq(q, s, gB))
            emit_mms(bseq)

            # ---------- activations + state update, per pass ----------
            for pas, qs, g in (("A", actA, gA), ("B", actB, gB)):
                if not qs:
                    continue
                lo = min(_rows(q)[0] for q in qs)
                hi = max(_rows(q)[1] for q in qs)
                segs = [(32, 64), (64, hi)] if (lo == 32 and hi > 64) \
                    else [(lo, hi)]
                c0 = 0 if pas == "A" else 256
                s_ifo = apool.tile([128, 768], fp, name=f"sifo{pas}",
                                   tag=f"sifo{pas}")
                s_g = apool.tile([128, 256], fp, name=f"sg{pas}",
                                 tag=f"sg{pas}")
                tmp1 = apool.tile([128, 256], fp, name=f"tmp1{pas}",
                                  tag=f"tmp1{pas}")
                tmp2 = apool.tile([128, 256], fp, name=f"tmp2{pas}",
                                  tag=f"tmp2{pas}")
                thc = apool.tile([128, 256], fp, name=f"thc{pas}",
                                 tag=f"thc{pas}")
                for lo, hi in segs:
                    if pas == "A":
                        nc.vector.tensor_add(g[lo:hi, 0:1024],
                                             g[lo:hi, 0:1024],
                                             biasA_sb[lo:hi, 0:1024])
                    nc.scalar.activation(s_ifo[lo:hi, :], g[lo:hi, 0:768],
                                         AF.Sigmoid)
                    nc.scalar.activation(s_g[lo:hi, :], g[lo:hi, 768:1024],
                                         AF.Tanh)
                    nc.vector.tensor_mul(tmp1[lo:hi, :], s_ifo[lo:hi, 256:512],
                                         c_all[lo:hi, c0:c0 + 256])
                    nc.vector.tensor_mul(tmp2[lo:hi, :], s_ifo[lo:hi, 0:256],
                                         s_g[lo:hi, :])
                    nc.vector.tensor_add(c_all[lo:hi, c0:c0 + 256],
                                         tmp1[lo:hi, :], tmp2[lo:hi, :])
                    nc.scalar.activation(thc[lo:hi, :],
                                         c_all[lo:hi, c0:c0 + 256], AF.Tanh)
                    nc.vector.tensor_mul(h_tile[lo:hi, c0:c0 + 256],
                                         s_ifo[lo:hi, 512:768], thc[lo:hi, :])
                if pas == "A":
                    # A-transposes for next step's A-class (+ DVE copies)
                    for ki, k in ((0, 0), (1, 1)):
                        src = h_tile[:, 128 * ki:128 * ki + 128]
                        nc.tensor.transpose(tp_cur[:, 128 * ki:128 * ki + 128],
                                            src, id_sb[:])
                        dst = hTpool.tile([128, 128], bf, name=f"hTA{k}",
                                          tag=f"hTA{k}")
                        nc.vector.tensor_copy(
                            dst[:], tp_cur[:, 128 * ki:128 * ki + 128])
                        hT[("A", k)] = dst

            if s == S - 1 and smax is None:
                nc.sync.dma_start(out_d[:], h_tile[96:128, 256:384])

            h_prev, tp_prev, prev_actB = h_tile, tp_cur, actB

    nc.finalize()
    return nc


def _prep_weights(inputs):
    """Host-side: transpose/permute all weights into the kernel layouts."""
    p1 = _gate_perm(H)   # 1024
    p2 = _gate_perm(E)   # 512
    w = {}
    w["ident"] = np.eye(128, dtype=np.float32).astype(BF16)

    w_ih0_1 = np.asarray(inputs["w_ih0_1"], np.float32)   # [4H, F]
    w_ihr_1 = np.asarray(inputs["w_ihr_1"], np.float32)   # [NL-1, 4H, H]
    w_hh_1 = np.asarray(inputs["w_hh_1"], np.float32)     # [NL, 4H, H]
    b_1 = np.asarray(inputs["b_1"], np.float32)           # [NL, 4H]
    w_ih0_2 = np.asarray(inputs["w_ih0_2"], np.float32)   # [4E, H]
    w_ihr_2 = np.asarray(inputs["w_ihr_2"], np.float32)   # [NL-1, 4E, E]
    w_hh_2 = np.asarray(inputs["w_hh_2"], np.float32)     # [NL, 4E, E]
    b_2 = np.asarray(inputs["b_2"], np.float32)           # [NL, 4E]

    # stack1 layer 0: rows 0-7 = w.T, row 8 = bias (rides x's ones feature)
    w0 = np.empty((9, 1024), np.float32)
    w0[:8] = w_ih0_1.T[:, p1]
    w0[8] = b_1[0][p1]
    w["win0_0"] = w0.astype(BF16)
    for q in range(5):
        for k in range(2):
            w[f"whh{q}_{k}"] = np.ascontiguousarray(
                w_hh_1[q].T[128 * k:128 * (k + 1), p1]).astype(BF16)
        if q >= 1:
            for k in range(2):
                w[f"win{q}_{k}"] = np.ascontiguousarray(
                    w_ihr_1[q - 1].T[128 * k:128 * (k + 1), p1]).astype(BF16)
    for q in range(5, 10):
        l2 = q - 5
        if q == 5:
            for k in range(2):
                w[f"win{q}_{k}"] = np.ascontiguousarray(
                    w_ih0_2.T[128 * k:128 * (k + 1), p2]).astype(BF16)
        else:
            w[f"win{q}_0"] = np.ascontiguousarray(
                w_ihr_2[l2 - 1].T[:, p2]).astype(BF16)
        w[f"whh{q}_0"] = np.ascontiguousarray(w_hh_2[l2].T[:, p2]).astype(BF16)

    # pass A: full per-partition-row bias table for the DVE psum add
    # (q0's bias rides x's ones feature -> rows 0:32 = 0).
    biasA = np.zeros((128, 1024), np.float32)
    for q in (1, 2, 3):
        biasA[32 * q:32 * q + 32, :] = b_1[q][p1]
    w["biasA"] = biasA.astype(BF16)
    # pass B keeps the indicator matmul (it doubles as the zero-init of the
    # pair-slot columns no layer matmul covers).
    indB = np.zeros((4, 128), np.float32)
    bcatB = np.zeros((4, 1024), np.float32)
    indB[0, 0:32] = 1.0
    bcatB[0] = b_1[4][p1]
    for p in range(3):
        indB[p + 1, 32 * (p + 1):32 * (p + 2)] = 1.0
        ba = b_2[2 * p][p2]
        for c in range(4):
            bcatB[p + 1, 256 * c:256 * c + 128] = ba[128 * c:128 * c + 128]
        if 2 * p + 1 < 5:
            bb = b_2[2 * p + 1][p2]
            for c in range(4):
                bcatB[p + 1, 256 * c + 128:256 * c + 256] = \
                    bb[128 * c:128 * c + 128]
    w["indB"] = indB.astype(BF16)
    w["bcatB"] = bcatB.astype(BF16)
    return w


def _prep_xt(x_core, T):
    """x shard [32, T, 8] -> [9, T*32] transposed with ones row, bf16."""
    xt = np.ones((9, T * BSH), np.float32)
    xt[:8] = np.ascontiguousarray(x_core.transpose(2, 1, 0)).reshape(8, T * BSH)
    return xt.astype(BF16)


def build_in_maps(inputs, T=T_FULL):
    x = np.asarray(inputs["x"], np.float32).reshape(B, T_FULL, F)
    w = _prep_weights(inputs)
    maps = []
    for c in range(NCORES):
        m = dict(w)
        m["xT"] = _prep_xt(x[BSH * c:BSH * (c + 1), :T], T)
        maps.append(m)
    return maps


def kernel(**inputs):
    from concourse.bass_utils import run_bass_kernel_spmd

    x = np.asarray(inputs["x"], np.float32).reshape(B, T_FULL, F)
    w = _prep_weights(inputs)

    nc = build_nc(T_FULL)
    in_maps = []
    for c in range(NCORES):
        m = dict(w)
        m["xT"] = _prep_xt(x[BSH * c:BSH * (c + 1)], T_FULL)
        in_maps.append(m)
    res = run_bass_kernel_spmd(nc, in_maps, list(range(NCORES))).results
    out = np.concatenate([np.asarray(r["out"]) for r in res], axis=0)
    return out.astype(np.float32)

